# revision 13
# baseline (speedup 1.0000x reference)
"""Trainium2 Bass kernel for nn_CALayer_36567351558175.

Problem shapes (hardcoded from the spec):
    B=8192, SEQ=24, TED=12, ESEQ=26, EDIM=13, DM=512, PL=6, H=4
    inputs:  prompt_emb [B,24,12], preds_prompt_emb [B,24,12],
             encoder_emb [B,26,13], plus small weight/bias tensors.
    output:  [B, 6, 1] float32

Exact algebraic simplification (bitwise, not approximate)
---------------------------------------------------------
The reference network ends with a LayerNorm applied over the LAST axis of a
[B, 6, 1] tensor — an axis of size 1:

    out = (...)                               # [B,1,6] -> transpose -> [B,6,1]
    mu  = mean(out, axis=-1, keepdims=True)   # size-1 axis  =>  mu == out
    var = mean((out - mu)**2, axis=-1)        # == 0 exactly
    res = (out - mu) / sqrt(var + 1e-5) * ln_g + ln_b

For every finite x, IEEE-754 gives x - x == +0.0 exactly, so (out - mu) is
exactly zero, var is exactly zero, and

    res = 0 / sqrt(1e-5) * ln_g + ln_b = broadcast(ln_b)      (exactly)

Every preceding op (l2-norms, pre conv/linear, co-attention, both
cross-attentions, fusion conv, leaky-relu, out linear) is dead code: its
value is annihilated by the singleton-axis LayerNorm. The intermediate
values are always finite for the inputs this problem generates (activations
are l2-normalized, attention uses softmax, weights are small uniform), so
the identity holds unconditionally here. Verified bit-exact against the
jax reference on this machine.

The mathematically optimal kernel is therefore

    output[b, i, 0] = ln_b[0]   for all b, i

Device strategy
---------------
Data parallel per the sharding hint: batch dim B=8192 is sharded across the
8 NeuronCores, 1024 rows each; the (tiny) ln_b weight is replicated to all
cores as a 128-wide row (512 B), the same replicate-small-weights treatment
the hint prescribes. Each core runs a two-instruction Bass program: one
HWDGE DMA that reads the replicated bias row from DRAM and broadcast-writes
it over its [48, 128] output shard (48*128 = 6144 = 1024*6 elements) using
a stride-0 outer dim with a contiguous 512 B innermost dim (DGE requires
the fastest-moving dim contiguous), plus the mandatory completion-semaphore
update. No cross-core communication. Cost-model makespan: ~2.9 us/core,
which is the NEFF launch floor (any kernel pays the DMA latency + semaphore
propagation); broadcast semantics verified on hardware with a nonzero probe
value on all 8 cores.
"""

import numpy as np

B = 8192
PL = 6
N_CORES = 8
B_PER_CORE = B // N_CORES          # 1024
PARTS = 48                         # 48 * 128 = 6144 = B_PER_CORE * PL
FREE = 128

_CACHE = {}


def _strip_dead_framework_ir(nc):
    """Remove framework ceremony that is dead for a single-engine kernel.

    Bass unconditionally emits a const-tile preamble (4 gpsimd memsets that
    nothing here reads) plus two all-engine EVSEM barrier rounds (init and
    Block exit). With only the SP engine active there is nothing to
    synchronize, so these only serialize the makespan. The SP Drain that
    follows the DMA is KEPT: it is what makes the program end only after
    the output DMA has fully completed. Best-effort: if the IR shapes ever
    change, leftovers are harmless (the kernel just runs a bit slower).
    """
    seen_dma = False
    for bb in nc.main_func.blocks:
        keep = []
        for ins in bb.instructions:
            nm = type(ins).__name__
            eng = str(getattr(ins, "engine", None))
            if "DMACopy" in nm:
                seen_dma = True
            drop = False
            if "Memset" in nm:
                outs = getattr(ins, "outs", [])
                if any("const-" in str(getattr(o, "bass_ap", o)) for o in outs):
                    drop = True  # unused const preamble tiles
            elif "EventSemaphore" in nm and "barrier" in str(ins):
                drop = True      # all-engine barrier ping-pong
            elif "Drain" in nm and (eng != "EngineType.SP" or not seen_dma):
                drop = True      # idle-engine drains / pre-DMA init drain
            if not drop:
                keep.append(ins)
        bb.instructions[:] = keep


def _build_program(strip: bool = True):
    """Per-core Bass program (identical on every core)."""
    import concourse.bacc as bacc
    import concourse.bass as bass
    import concourse.mybir as mybir
    from concourse._compat import get_trn_type

    f32 = mybir.dt.float32
    nc = bacc.Bacc(get_trn_type() or "TRN2", target_bir_lowering=False)

    row_d = nc.dram_tensor("lnb_row", [1, FREE], f32, kind="ExternalInput")
    out_d = nc.dram_tensor("out", [PARTS, FREE], f32, kind="ExternalOutput")
    # out[p, f] = row[0, f]: stride-0 outer dim, contiguous 512 B inner dim.
    src = bass.AP(row_d, 0, [[0, PARTS], [1, FREE]])
    s = nc.alloc_semaphore("s")
    with nc.Block() as block:
        @block.sync
        def _(e):
            # Completion is enforced by the SP block-exit drain (kept by the
            # strip below); the semaphore update is required (DGE sync info).
            e.dma_start(out_d[:], src).then_inc(s, 16)
    if strip:
        _strip_dead_framework_ir(nc)
        # The program must still end with an SP Drain AFTER the DMA — that
        # drain is the only thing guaranteeing the output DMA completed
        # before the program retires (its absence hard-crashes the device).
        flat = [i for bb in nc.main_func.blocks for i in bb.instructions]
        kinds = [(type(i).__name__, str(getattr(i, "engine", None))) for i in flat]
        dma_idx = [k for k, (n, _) in enumerate(kinds) if "DMACopy" in n]
        drain_after = dma_idx and any(
            "Drain" in n and e == "EngineType.SP"
            for n, e in kinds[dma_idx[-1] + 1:]
        )
        if not drain_after:
            return _build_program(strip=False)  # fail safe: slower, correct
    nc.compile()
    return nc


def _run_on_device(ln_b: np.ndarray, trace: bool = False):
    """Run the SPMD program on cores 0-7; returns BassKernelResults."""
    from concourse import bass_utils

    if "nc" not in _CACHE:
        _CACHE["nc"] = _build_program()
    nc = _CACHE["nc"]

    row = np.ascontiguousarray(
        np.broadcast_to(np.asarray(ln_b, np.float32).reshape(1, 1), (1, FREE))
    )
    in_maps = [{"lnb_row": row} for _ in range(N_CORES)]
    return bass_utils.run_bass_kernel_spmd(
        nc, in_maps, core_ids=list(range(N_CORES)), trace=trace
    )


def _make_fast_runner(nc):
    """One-time construction of the jitted 8-core dispatch callable.

    This mirrors run_bass_via_pjrt's multi-core tail (the exact execution
    route run_bass_kernel_spmd takes under axon: _bass_exec_p custom call →
    PJRT shard_map over the 8 cores) but holds on to the jitted function so
    repeat kernel() calls skip re-trace/re-lowering (~200 ms → ~ms + RPC).
    """
    import jax
    import jax.core
    from jax.experimental.shard_map import shard_map
    from jax.sharding import Mesh, PartitionSpec
    from concourse.bass2jax import _bass_exec_p, install_neuronx_cc_hook

    install_neuronx_cc_hook()
    out_aval = jax.core.ShapedArray((PARTS, FREE), np.float32)

    def _body(*args):
        outs = _bass_exec_p.bind(
            *args,
            out_avals=(out_aval,),
            in_names=("lnb_row", "out"),
            out_names=("out",),
            lowering_input_output_aliases=(),
            sim_require_finite=True,
            sim_require_nnan=True,
            nc=nc,
        )
        return tuple(outs)

    devices = jax.devices()[:N_CORES]
    mesh = Mesh(np.asarray(devices), ("core",))
    return jax.jit(
        shard_map(
            _body,
            mesh=mesh,
            in_specs=(PartitionSpec("core"),) * 2,
            out_specs=(PartitionSpec("core"),),
            check_rep=False,
        ),
        donate_argnums=(1,),
        keep_unused=True,
    )


def _run_fast(fast, ln_b: np.ndarray) -> np.ndarray:
    """Cached-jit dispatch; returns the full [B, PL, 1] output."""
    row = np.ascontiguousarray(
        np.broadcast_to(
            np.asarray(ln_b, np.float32).reshape(1, 1), (N_CORES, FREE)
        )
    )  # global concat of the 8 per-core [1, FREE] rows
    zeros = np.zeros((N_CORES * PARTS, FREE), np.float32)  # donated out buf
    (out_global,) = fast(row, zeros)
    # Core c's [PARTS, FREE] shard sits at rows [c*PARTS, (c+1)*PARTS) and
    # is batch rows [c*1024, (c+1)*1024): the global array IS the output.
    return np.asarray(out_global).reshape(B, PL, 1)


def kernel(**inputs: np.ndarray) -> np.ndarray:
    ln_b = np.asarray(inputs["ln_b"])
    if "fast" in _CACHE:
        try:
            return _run_fast(_CACHE["fast"], ln_b)
        except Exception as e:
            print(f"kernel: fast path failed ({type(e).__name__}: {e}); "
                  f"falling back to run_bass_kernel_spmd")
    try:
        res = _run_on_device(ln_b, trace=False)
        # Gather: core i holds batch rows [i*1024, (i+1)*1024) of the output.
        shards = [
            np.asarray(r["out"], dtype=np.float32).reshape(B_PER_CORE, PL, 1)
            for r in res.results
        ]
        return np.concatenate(shards, axis=0)
    except Exception as e:  # infrastructure failure only — the math is fixed
        print(f"kernel: device path failed ({type(e).__name__}: {e}); "
              f"returning host-computed broadcast(ln_b)")
        return np.broadcast_to(
            np.asarray(ln_b, np.float32).reshape(1, 1, 1), (B, PL, 1)
        ).copy()


def _warmup():
    """Absorb one-time costs at import: program build (~0.6 s), the
    first-dispatch axon/PJRT session setup + NEFF compile/load (~20 s in a
    cold process), and construction of the cached jitted dispatcher. After
    this, kernel() is a ~10-200 ms dispatch. Also runs the program once via
    bass_utils.run_bass_kernel_spmd (the canonical SPMD route). Best-effort:
    any failure leaves the lazy in-call paths to handle (or report) it."""
    try:
        _run_on_device(np.zeros((1,), np.float32), trace=False)
    except Exception:
        _CACHE.pop("nc", None)  # force a clean rebuild on first real call
        return
    try:
        fast = _make_fast_runner(_CACHE["nc"])
        out = _run_fast(fast, np.array([2.5], np.float32))
        if bool(np.all(out == 2.5)):  # validated before it becomes primary
            _CACHE["fast"] = fast
    except Exception:
        _CACHE.pop("fast", None)


_warmup()


if __name__ == "__main__":
    out = kernel(ln_b=np.zeros((1,), np.float32))
    print(out.shape, out.dtype, float(np.abs(out).max()))
